# revision 4
# baseline (speedup 1.0000x reference)
"""Haar-DWT downsampling + 1x1 conv + BN + ReLU fused Trainium2 kernel.

Math: the Haar DWT (J=1) followed by a 1x1 conv over the 4C subband
channels, inference BN, and ReLU is one linear op + bias + ReLU.  It
folds into a 2x2/stride-2 conv:

    z[o, i, j] = relu( sum_{c,di,dj} Weff[o, c, di, dj] * x[c, 2i+di, 2j+dj]
                       + bias_total[o] )

with Weff/bias_total computed on the host from (W, b, gamma, beta, mean,
var).  On-device this is, per output tile, accumulating matmuls + one
DVE pass (bias + ReLU) reading PSUM.

Precision: x and the folded weights are cast to bf16 on the host and z
is produced as bf16 (upcast to f32 on the host).  This halves HBM
traffic on both sides (the kernel is HBM-bound) and doubles PE
throughput; measured end-to-end rel err ~4e-3 vs the 2e-2 gate.

Sharding: pure data-parallel over batch. B=16 -> 2 images per core on
8 cores. Each core reads only its x shard and writes only its z shard
(16.8 MB in + 8.4 MB out per core in bf16).

DMA note: SDMA sprays a transfer's descriptors over the 16 engines by
outermost-AP-dim index, so every transfer here keeps a >=64 outer dim
(c or o); an outer dim of 2 would serialize onto 2 engines.
"""

import numpy as np
import ml_dtypes

import concourse.bass as bass
import concourse.bacc as bacc
import concourse.mybir as mybir
from concourse.tile import TileContext
from concourse.bass_utils import run_bass_kernel_spmd

BN_EPS = 1e-5

# Problem shape (hardcoded per harness contract)
B, C, H, W_IMG = 16, 64, 256, 256
COUT = 128
N_CORES = 8
B_LOCAL = B // N_CORES          # 2 images per core
HO, WO = H // 2, W_IMG // 2     # 128 x 128 output image

F32 = mybir.dt.float32
BF16 = mybir.dt.bfloat16
NP_BF16 = ml_dtypes.bfloat16


def _fold_weights(W, b, gamma, beta, mean, var):
    """Fold DWT + conv + BN into per-(di,dj) lhsT weights
    [4, 128(K), 128(M=o)] and a per-channel bias [COUT].

    Combo q = di*2 + dj.  K rows 0-63 and 64-127 hold the SAME c-indexed
    weights (duplicated): the kernel packs two K=64 matmuls into the PE
    array (partition halves 0/64), one per h-half of the input tile, and
    lhsT/rhs base partitions must match.
    """
    W = W.astype(np.float64)
    Wll, Wlh, Whl, Whh = W[:, :C], W[:, C:2 * C], W[:, 2 * C:3 * C], W[:, 3 * C:]
    s = (gamma.astype(np.float64) / np.sqrt(var.astype(np.float64) + BN_EPS))
    coef = {
        (0, 0): 0.5 * (Wll + Wlh + Whl + Whh),
        (0, 1): 0.5 * (Wll + Wlh - Whl - Whh),
        (1, 0): 0.5 * (Wll - Wlh + Whl - Whh),
        (1, 1): 0.5 * (Wll - Wlh - Whl + Whh),
    }
    bias_total = (b.astype(np.float64) * s + beta.astype(np.float64)
                  - mean.astype(np.float64) * s)
    lhsT = np.zeros((4, 128, COUT), dtype=np.float64)
    for di in range(2):
        for dj in range(2):
            wq = (coef[(di, dj)] * s[:, None]).T   # [c, o]
            lhsT[di * 2 + dj, 0:C, :] = wq
            lhsT[di * 2 + dj, C:2 * C, :] = wq
    return lhsT.astype(NP_BF16), bias_total.astype(np.float32)


def build_nc(b_local=B_LOCAL, n_row_blocks=2, run_bacc_compile=True):
    """n_row_blocks: 64-output-row blocks per image (full image = 2)."""
    nc = bacc.Bacc(None)
    x = nc.dram_tensor("x", [b_local, C, H, W_IMG], BF16, kind="ExternalInput")
    w_lhsT = nc.dram_tensor("w_lhsT", [4, 128, COUT], BF16, kind="ExternalInput")
    bias = nc.dram_tensor("bias", [COUT, 1], F32, kind="ExternalInput")
    z = nc.dram_tensor("z", [b_local, COUT, HO, WO], BF16, kind="ExternalOutput")

    with TileContext(nc) as tc:
        with (
            tc.tile_pool(name="consts", bufs=1) as cpool,
            tc.tile_pool(name="xin", bufs=3) as xpool,
            tc.tile_pool(name="psum", bufs=2, space="PSUM") as ppool,
            tc.tile_pool(name="zout", bufs=2) as zpool,
        ):
            w_sb = []
            for q in range(4):
                wt = cpool.tile([128, COUT], BF16, name=f"w{q}_sb")
                nc.sync.dma_start(out=wt[:], in_=w_lhsT[q])
                w_sb.append(wt)
            bias_sb = cpool.tile([COUT, 1], F32)
            nc.sync.dma_start(out=bias_sb[:], in_=bias[:])

            for bi in range(b_local):
                for tb in range(n_row_blocks):
                    # 128 input rows -> 64 output rows; partition =
                    # (hhalf, c): each partition holds 64 contiguous
                    # input rows (32 KB bf16).  One DMA per hhalf so
                    # the outer AP dim is c=64 (full 16-engine spray).
                    xt = xpool.tile([128, 64 * W_IMG], BF16)
                    for hh in range(2):
                        src = x[bi, :, 128 * tb + 64 * hh:
                               128 * tb + 64 * (hh + 1), :].rearrange(
                            "c hl w -> c (hl w)"
                        )
                        nc.sync.dma_start(
                            out=xt[64 * hh:64 * (hh + 1), :], in_=src)
                    # free f = il*512 + di*256 + j*2 + dj  (il<32 per half)
                    xv = xt.rearrange(
                        "p (il di j dj) -> p di dj il j", di=2, j=WO, dj=2
                    )
                    # One output tile for the whole 64-row block, laid
                    # out exactly as z[bi, :, 64tb:64tb+64, :] so the
                    # store is one DMA with 16 KB-contiguous partitions.
                    # f_out = (32h + 8pt + r)*128 + j = h*4096 + pt*1024
                    #         + r*128 + j
                    zt = zpool.tile([COUT, 8192], BF16)
                    zv = zt.rearrange("o (h g f) -> o h g f", h=2, g=4)
                    for pt in range(4):   # four psum tiles per block
                        ps = ppool.tile([COUT, 2048], F32)
                        # psum f = h*1024 + gg*512 + (il-il0)*128 + j,
                        # covering rows (64tb + 32h + 8pt + 4gg + 0..3)
                        for q in range(4):
                            di, dj = q // 2, q % 2
                            for h in range(2):
                                lw = w_sb[q][h * C:(h + 1) * C, :]
                                for gg in range(2):
                                    il0 = 8 * pt + 4 * gg
                                    nc.tensor.matmul(
                                        ps[:, h * 1024 + gg * 512:
                                           h * 1024 + gg * 512 + 512],
                                        lhsT=lw,
                                        rhs=xv[h * C:(h + 1) * C, di, dj,
                                               il0:il0 + 4, :],
                                        start=(q == 0),
                                        stop=(q == 3),
                                    )
                        # bias + ReLU in one DVE pass: max(ps + bias, 0)
                        nc.vector.tensor_scalar(
                            zv[:, :, pt, :],
                            ps.rearrange("o (h f) -> o h f", h=2),
                            bias_sb[:, 0:1], 0.0,
                            mybir.AluOpType.add, mybir.AluOpType.max,
                        )
                    nc.sync.dma_start(
                        out=z[bi, :, 64 * tb:64 * (tb + 1), :].rearrange(
                            "o h w -> o (h w)"),
                        in_=zt[:],
                    )
    if run_bacc_compile:
        nc.compile()
    return nc


_NC_CACHE = {}


def _get_nc():
    if "nc" not in _NC_CACHE:
        _NC_CACHE["nc"] = build_nc()
    return _NC_CACHE["nc"]


def kernel(x, W, b, gamma, beta, mean, var, _trace=False):
    x = np.ascontiguousarray(
        np.asarray(x, dtype=np.float32).astype(NP_BF16))
    lhsT, bias_total = _fold_weights(
        np.asarray(W), np.asarray(b), np.asarray(gamma),
        np.asarray(beta), np.asarray(mean), np.asarray(var),
    )
    bias_col = np.ascontiguousarray(bias_total.reshape(COUT, 1))

    nc = _get_nc()
    in_maps = []
    for core in range(N_CORES):
        xs = np.ascontiguousarray(x[core * B_LOCAL:(core + 1) * B_LOCAL])
        in_maps.append({"x": xs, "w_lhsT": lhsT, "bias": bias_col})

    res = run_bass_kernel_spmd(
        nc, in_maps, list(range(N_CORES)), trace=_trace
    )
    out = np.concatenate(
        [res.results[i]["z"] for i in range(N_CORES)], axis=0
    ).astype(np.float32)
    if _trace:
        return out, res
    return out
